# revision 40
# baseline (speedup 1.0000x reference)
"""Multi-head attention (B=2, S=2048, E=1024, H=16) on 8 Trainium2 NeuronCores.

Sharding: tensor-parallel over heads — core i owns heads (2i, 2i+1).
Token ownership for the output projection: core i owns tokens
[256*i, 256*i+256) of EACH batch, so the head->token re-shard can be done
with one AllToAll PER BATCH and overlapped with the other batch's compute.

  Phase A  (per core, per batch): q/k/v projections for its 2 heads,
            feature-major into single qfm/kfm tiles (head A on partitions
            0:64, head B on 64:128); v is PE-transposed to token-major with
            a ones column per head (softmax-denominator trick).
  Phase B/C (per core, per batch): scores^T via PE row-tiled K=64 matmul
            pairs at N=512 (head A on array rows 0:64 via tile_position=
            (0,0), head B on rows 64:128 via (64,0) — no zero-padding
            waste); exp straight out of PSUM (no max-subtraction — scores
            are O(1)), split between ScalarE (exact) and DVE (Schraudolph
            bitcast exp into bf16 bit-space, head B on even q-tiles) to
            balance the two engines; AV matmul per 256-col half with the
            ones-row so the denominator falls out of the same fp32
            accumulation (head A's kc=0 start resets the shared PSUM bank,
            zeroing head B's region — bank-wide start semantics); normalize
            via a rank-1 f32r (ones ⊗ den) broadcast matmul +
            reciprocal_approx_fast on the broadcast tile (custom-DVE ops
            only work at base partition 0) — no GpSimd or DMA on the
            per-q-tile path, since the in-flight AllToAll blocks the GpSimd
            queue and would stall it.
  AllToAll: one bf16 collective per batch re-sharding head-parallel
            [128 feat, S tokens] to token-parallel [1024 feat, S/8 tokens].
            Batch 0's collective overlaps phase A/BC of batch 1.
  Phase D  (per core, per batch): output projection for its 256-token slice;
            its PSUM comes from the BC-phase rings so the WAR dependency
            pins the matmuls behind the preceding batch's BC work — the
            scheduler otherwise hoists them into the in-order PE queue where
            they head-of-line block on the collective's output. D(b0)
            overlaps a2a(b1).

Phase A of batch 1 is emitted interleaved into BC(b0)'s q-tiles so the PE
fills ScalarE-bound gaps. Matmuls run in bf16 (full-rate + FWL weight
loads); inputs are cast to bf16 on the host; PSUM accumulation is fp32.

Phase A(0) leads with small (128/256-token) chunks so the first
projection matmuls start after a 256KB xt transfer instead of 1MB.

Measured on 8 axon trn2 cores: ~307-311 us HW exec (mean core ~300-306),
rel err 7.4e-3 (vs 402 us / 3.1e-3 for the padded-K, single-collective,
serial-tail baseline).
"""

import numpy as np
import ml_dtypes

import concourse.bass as bass
import concourse.mybir as mybir
import concourse.tile as tile
from concourse import bacc
from concourse import bass_utils
from concourse.masks import make_identity

F32 = mybir.dt.float32
BF16 = mybir.dt.bfloat16
F32R = mybir.dt.float32r
I16 = mybir.dt.int16
N_CORES = 8
P = 128

COMPUTE = "bf16"
SCORES_TILED = True           # row-tiled K=64 score matmul pairs
EXP_B_DVE = True             # route head-B exp to DVE (Schraudolph bf16)

# Full problem dims (hardcoded per the harness contract)
B_FULL, S_FULL, E, H, D = 2, 2048, 1024, 16, 64
HPC = H // N_CORES            # heads per core = 2
F = HPC * D                   # feature cols per core = 128
SCALE = D ** -0.5

# Schraudolph exp in bf16-bit space: w = bitcast_bf16(int16(round(
#   s * SCALE*log2e*128 + (127<<7) - C)))
LOG2E = 1.4426950408889634
SCH_A = SCALE * LOG2E * 128.0
SCH_B = 16256.0 - 29.0        # assumes round-to-nearest convert


def build_nc(B=B_FULL, S=S_FULL, compute=COMPUTE):
    CDT = BF16 if compute == "bf16" else F32R
    IN_DT = BF16 if compute == "bf16" else F32
    T = B * S                 # tokens
    KO = E // P               # 8 contraction chunks over embed
    TC = min(512, S)          # phase-A token chunk
    NTC = S // TC             # chunks per batch
    Q2 = min(512, S)          # q tile (score matmul N)
    NQ = S // Q2
    QH = min(256, Q2)         # AV/normalize half-tile
    NH = Q2 // QH
    KC = S // P               # k chunks per batch
    G4 = 2                    # kc group per exp call
    TPB = S // N_CORES        # tokens per core PER BATCH
    TT = min(P, TPB)

    nc = bacc.Bacc("TRN2", target_bir_lowering=False, debug=False,
                   num_devices=N_CORES)

    xT = nc.dram_tensor("xT", [E, T], IN_DT, kind="ExternalInput").ap()
    wq = nc.dram_tensor("wq", [E, F], IN_DT, kind="ExternalInput").ap()
    wk = nc.dram_tensor("wk", [E, F], IN_DT, kind="ExternalInput").ap()
    wv = nc.dram_tensor("wv", [E, F], IN_DT, kind="ExternalInput").ap()
    bq = nc.dram_tensor("bq", [F, 1], F32, kind="ExternalInput").ap()
    bk = nc.dram_tensor("bk", [F, 1], F32, kind="ExternalInput").ap()
    bv = nc.dram_tensor("bv", [F, 1], F32, kind="ExternalInput").ap()
    ow = nc.dram_tensor("ow", [E, E], IN_DT, kind="ExternalInput").ap()
    ob = nc.dram_tensor("ob", [1, E], F32, kind="ExternalInput").ap()
    # rows b*TPB+i = batch b, token TPB*core + i
    out = nc.dram_tensor("out", [B * TPB, E], F32, kind="ExternalOutput").ap()

    Exp = mybir.ActivationFunctionType.Exp
    Mult = mybir.AluOpType.mult
    Add = mybir.AluOpType.add

    with tile.TileContext(nc) as tc:
        with tc.tile_pool(name="persist", bufs=1) as persist, \
             tc.tile_pool(name="pAw", bufs=1) as pAw, \
             tc.tile_pool(name="pA", bufs=4) as pA, \
             tc.tile_pool(name="pBC", bufs=2) as pBC, \
             tc.tile_pool(name="pNr", bufs=3) as pNr, \
             tc.tile_pool(name="pD", bufs=1) as pD, \
             tc.tile_pool(name="pDo", bufs=4) as pDo, \
             tc.tile_pool(name="psA", bufs=1, space="PSUM") as psA, \
             tc.tile_pool(name="psS", bufs=2, space="PSUM") as psS, \
             tc.tile_pool(name="psAV", bufs=2, space="PSUM") as psAV, \
             tc.tile_pool(name="psDb", bufs=1, space="PSUM") as psDb, \
             tc.tile_pool(name="dramp", bufs=1, space="DRAM") as dramp:
            ident = persist.tile([P, P], CDT)
            make_identity(nc, ident)
            bq_sb = persist.tile([P, 1], F32)
            bk_sb = persist.tile([P, 1], F32)
            bv_sb = persist.tile([P, 1], F32)
            nc.sync.dma_start(bq_sb, bq)
            nc.sync.dma_start(bk_sb, bk)
            nc.sync.dma_start(bv_sb, bv)
            ob_row = persist.tile([1, E], F32)
            nc.sync.dma_start(ob_row, ob)
            obb = persist.tile([P, E], F32)
            nc.gpsimd.partition_broadcast(obb, ob_row)

            qfm = persist.tile([P, T], CDT)     # q^T (both heads stacked)
            if SCORES_TILED:
                kfm = persist.tile([P, T], CDT)  # k^T (both heads stacked)
            else:
                # zero-padded per head: full-K=128 standard matmuls
                kfmA = persist.tile([P, T], CDT)
                kfmB = persist.tile([P, T], CDT)
                nc.vector.memset(kfmA[64:128], 0.0)
                nc.vector.memset(kfmB[0:64], 0.0)
            # v token-major per 128-token chunk, with a ones column per head:
            # cols 0:64 head A v, col 64 ones(A), 65:129 head B v, col 129
            # ones(B)
            vtm = persist.tile([P, T // P, 130], CDT)
            ones1 = persist.tile([P, 1], F32)
            nc.vector.memset(ones1, 1.0)
            nc.vector.tensor_copy(vtm[:, :, 64], ones1.to_broadcast([P, T // P]))
            nc.vector.tensor_copy(vtm[:, :, 129], ones1.to_broadcast([P, T // P]))

            # f32r constants for the rank-1 normalize matmuls
            ones_f32 = persist.tile([P, 65], F32)
            nc.vector.memset(ones_f32, 1.0)
            ones_bc = persist.tile([P, 65], F32R)
            nc.vector.tensor_copy(ones_bc, ones_f32)

            wq_sb = pAw.tile([P, KO, F], CDT)
            wk_sb = pAw.tile([P, KO, F], CDT)
            wv_sb = pAw.tile([P, KO, F], CDT)
            wqr = wq.rearrange("(ko p) f -> p ko f", p=P)
            for ko in range(KO):
                nc.sync.dma_start(wq_sb[:, ko], wqr[:, ko])
            xTr = xT.rearrange("(ko p) t -> p ko t", p=P)
            wkv_loaded = []

            a2a_in = [nc.dram_tensor(f"a2a_in{b}", [N_CORES, P, TPB], CDT,
                                     kind="Internal").ap()
                      for b in range(B)]
            a2a_out = [nc.dram_tensor(f"a2a_out{b}", [N_CORES, P, TPB], CDT,
                                      kind="Internal").ap()
                       for b in range(B)]

            def phase_a_qk(b, t0, tc):
                t0 = b * S + t0
                xt = pA.tile([P, KO, tc], CDT, tag="xt")
                nc.sync.dma_start(xt, xTr[:, :, t0:t0 + tc])
                if not wkv_loaded:
                    nc.sync.dma_start(
                        wk_sb, wk.rearrange("(ko p) f -> p ko f", p=P))
                    nc.sync.dma_start(
                        wv_sb, wv.rearrange("(ko p) f -> p ko f", p=P))
                    wkv_loaded.append(True)
                ps = psA.tile([P, tc], F32, tag="ps")
                for ko in range(KO):
                    nc.tensor.matmul(ps, lhsT=wq_sb[:, ko], rhs=xt[:, ko],
                                     start=(ko == 0), stop=(ko == KO - 1))
                nc.vector.tensor_scalar_add(qfm[:, t0:t0 + tc], ps, bq_sb)
                ps = psA.tile([P, tc], F32, tag="ps")
                for ko in range(KO):
                    nc.tensor.matmul(ps, lhsT=wk_sb[:, ko], rhs=xt[:, ko],
                                     start=(ko == 0), stop=(ko == KO - 1))
                if SCORES_TILED:
                    nc.vector.tensor_scalar_add(kfm[:, t0:t0 + tc], ps, bk_sb)
                else:
                    nc.vector.tensor_scalar_add(kfmA[0:64, t0:t0 + tc],
                                                ps[0:64], bk_sb[0:64])
                    nc.vector.tensor_scalar_add(kfmB[64:128, t0:t0 + tc],
                                                ps[64:128], bk_sb[64:128])
                return xt

            def phase_a_v(b, t0, tc, xt):
                t0 = b * S + t0
                ps = psA.tile([P, tc], F32, tag="ps")
                for ko in range(KO):
                    nc.tensor.matmul(ps, lhsT=wv_sb[:, ko], rhs=xt[:, ko],
                                     start=(ko == 0), stop=(ko == KO - 1))
                vfm = pA.tile([P, tc], CDT, tag="vfm")
                nc.vector.tensor_scalar_add(vfm, ps, bv_sb)
                for sub in range(tc // P):
                    pst = psDb.tile([P, P], CDT, tag="db")
                    nc.tensor.transpose(pst, vfm[:, sub * P:(sub + 1) * P],
                                        ident)
                    c = (t0 + sub * P) // P
                    nc.vector.tensor_copy(vtm[:, c, 0:64], pst[:, 0:64])
                    nc.vector.tensor_copy(vtm[:, c, 65:129], pst[:, 64:128])

            def phase_a(b, chunks=None):
                t0 = 0
                for tc in (chunks or [TC] * NTC):
                    xt = phase_a_qk(b, t0, tc)
                    phase_a_v(b, t0, tc, xt)
                    t0 += tc

            def phase_a_pieces(b):
                t0 = 0
                for tc in [TC] * NTC:
                    xt = phase_a_qk(b, t0, tc)
                    yield
                    phase_a_v(b, t0, tc, xt)
                    yield
                    t0 += tc

            def bc_qtile(b, qi):
                q0 = b * S + qi * Q2
                eA = pBC.tile([P, KC, Q2], CDT, tag="expA")
                eB = pBC.tile([P, KC, Q2], CDT, tag="expB")
                for kg in range(KC // G4):
                    sA = psS.tile([P, G4, Q2], F32, tag="sS")
                    sB = psS.tile([P, G4, Q2], F32, tag="sS")
                    for j in range(G4):
                        kc = kg * G4 + j
                        k0 = b * S + kc * P
                        if SCORES_TILED:
                            nc.tensor.matmul(
                                sA[:, j], lhsT=kfm[0:64, k0:k0 + P],
                                rhs=qfm[0:64, q0:q0 + Q2],
                                start=True, stop=True, tile_position=(0, 0))
                            nc.tensor.matmul(
                                sB[:, j], lhsT=kfm[64:128, k0:k0 + P],
                                rhs=qfm[64:128, q0:q0 + Q2],
                                start=True, stop=True, tile_position=(64, 0))
                        else:
                            nc.tensor.matmul(
                                sA[:, j], lhsT=kfmA[:, k0:k0 + P],
                                rhs=qfm[:, q0:q0 + Q2],
                                start=True, stop=True)
                            nc.tensor.matmul(
                                sB[:, j], lhsT=kfmB[:, k0:k0 + P],
                                rhs=qfm[:, q0:q0 + Q2],
                                start=True, stop=True)
                    g0 = kg * G4
                    nc.scalar.activation(eA[:, g0:g0 + G4], sA, Exp,
                                         scale=SCALE)
                    if EXP_B_DVE and qi % 2 == 0:
                        nc.vector.tensor_scalar(
                            eB[:, g0:g0 + G4].bitcast(I16), sB,
                            SCH_A, SCH_B, op0=Mult, op1=Add)
                    else:
                        nc.scalar.activation(eB[:, g0:g0 + G4], sB, Exp,
                                             scale=SCALE)
                for h in range(NH):
                    bc_avh(b, qi, h, eA, eB)

            def bc_avh(b, qi, h, eA, eB):
                qh0 = h * QH
                pv = psAV.tile([65, 2, QH], F32, tag="av")
                pvA = pv[:, 0]
                pvB = pv[:, 1]
                # chain A's kc=0 start resets the ENTIRE shared PSUM bank
                # (bank-wide start semantics), zeroing chain B's region too;
                # everything after accumulates with start=False
                for kc in range(KC):
                    c = (b * S) // P + kc
                    nc.tensor.matmul(pvA, lhsT=vtm[:, c, 0:65],
                                     rhs=eA[:, kc, qh0:qh0 + QH],
                                     start=(kc == 0), stop=False,
                                     skip_group_check=True)
                    nc.tensor.matmul(pvB, lhsT=vtm[:, c, 65:130],
                                     rhs=eB[:, kc, qh0:qh0 + QH],
                                     start=False, stop=(kc == KC - 1),
                                     skip_group_check=True)
                # normalize: rows 0:64 numerator, row 64 denominator.
                # reciprocal on the psum den rows, then broadcast down 64
                # partitions with a rank-1 (ones ⊗ recip) matmul.
                # gpsimd-free normalize: broadcast the raw denominator rows
                # down 64 partitions with a rank-1 (ones ⊗ den) matmul, then
                # reciprocal on the broadcast tile (custom-DVE ops only work
                # at base partition 0). Keeps the per-q-tile path off the
                # GpSimd queue (the AllToAll blocks that queue in flight).
                rr2 = pNr.tile([65, 2, QH], F32R, tag="rr2")
                nc.vector.tensor_copy(rr2[64:65], pv[64:65])
                db = psDb.tile([64, 2, QH], F32, tag="db")
                nc.tensor.matmul(db,
                                 lhsT=ones_bc[64:65, 0:64],
                                 rhs=rr2[64:65],
                                 start=True, stop=True,
                                 tile_position=(64, 0))
                dbs = pNr.tile([64, 2, QH], F32, tag="dbs")
                nc.vector.tensor_copy(dbs, db)
                nc.vector.reciprocal_approx_fast(dbs, dbs)
                stgA = pNr.tile([64, QH], CDT, tag="stgA")
                stgB = pNr.tile([64, QH], CDT, tag="stgB")
                nc.vector.tensor_mul(stgA, pvA[0:64], dbs[:, 0])
                nc.vector.tensor_mul(stgB, pvB[0:64], dbs[:, 1])
                dest = qi * NH + h
                nc.sync.dma_start(a2a_in[b][dest, 0:64], stgA)
                nc.sync.dma_start(a2a_in[b][dest, 64:128], stgB)

            def send_a2a(b):
                nc.gpsimd.collective_compute(
                    "AllToAll", mybir.AluOpType.bypass,
                    replica_groups=[list(range(N_CORES))],
                    ins=[a2a_in[b].opt()], outs=[a2a_out[b].opt()])

            ow_sb = pD.tile([P, KO, E], CDT)

            def d_block(ga, tslice, row0, nrows):
                # output-projection for ga[:, :, tslice] -> out rows
                # [row0, row0+nrows); PSUM from the BC rings so the WAR dep
                # pins these matmuls behind the preceding BC work in the
                # in-order PE queue (the scheduler otherwise hoists them to
                # block head-of-line on the collective's output)
                for n2 in range(E // 512):
                    if n2 % 2 == 0:
                        pso = psDb.tile([P, 512], F32, tag="db")
                    else:
                        pso = psS.tile([P, 512], F32, tag="sS")
                    for r in range(N_CORES):
                        nc.tensor.matmul(
                            pso[0:nrows],
                            lhsT=ga[:, r, tslice],
                            rhs=ow_sb[:, r, n2 * 512:(n2 + 1) * 512],
                            start=(r == 0), stop=(r == N_CORES - 1))
                    osb = pDo.tile([TT, 512], F32, tag="osb")
                    nc.vector.tensor_add(osb[0:nrows], pso[0:nrows],
                                         obb[0:nrows,
                                             n2 * 512:(n2 + 1) * 512])
                    nc.sync.dma_start(
                        out[row0:row0 + nrows, n2 * 512:(n2 + 1) * 512],
                        osb[0:nrows])

            def phase_d(b):
                ga = pD.tile([P, N_CORES, TPB], CDT, name=f"ga{b}")
                gar = a2a_out[b].rearrange("c p t -> p c t")
                for r in range(N_CORES):
                    nc.sync.dma_start(ga[:, r], gar[:, r])
                for t4 in range(TPB // TT):
                    d_block(ga, slice(t4 * TT, (t4 + 1) * TT),
                            b * TPB + t4 * TT, TT)

            # small leading chunks: the first q-projection matmuls start
            # after a 256KB xt transfer instead of 1MB
            phase_a(0, chunks=[128, 128, 256, 512, 512, 512])
            if B > 1:
                # interleave A(b1) pieces into BC(b0)'s scalar-bound q-tiles
                a1 = phase_a_pieces(1)
                for qi in range(NQ):
                    bc_qtile(0, qi)
                    next(a1, None)
                    next(a1, None)
                send_a2a(0)
                nc.sync.dma_start(ow_sb,
                                  ow.rearrange("(r p) e -> p r e", p=P))
                for qi in range(NQ):
                    bc_qtile(1, qi)
                phase_d(0)
                send_a2a(1)
                phase_d(1)
            else:
                for qi in range(NQ):
                    bc_qtile(0, qi)
                send_a2a(0)
                nc.sync.dma_start(ow_sb,
                                  ow.rearrange("(r p) e -> p r e", p=P))
                phase_d(0)

    nc.compile()
    return nc


def make_in_maps(x, qkv_w, qkv_b, o_w, o_b, B=B_FULL, S=S_FULL,
                 compute=COMPUTE):
    """Host-side sharding: full inputs -> per-core input dicts."""
    T = B * S
    idt = ml_dtypes.bfloat16 if compute == "bf16" else np.float32
    x = np.asarray(x, dtype=np.float32)
    qkv_w = np.asarray(qkv_w, dtype=np.float32).astype(idt)
    qkv_b = np.asarray(qkv_b, dtype=np.float32)
    o_w = np.ascontiguousarray(np.asarray(o_w, dtype=np.float32).astype(idt))
    o_b = np.asarray(o_b, dtype=np.float32).reshape(1, E)
    xT = np.ascontiguousarray(x.reshape(T, E).T.astype(idt))
    in_maps = []
    for i in range(N_CORES):
        c0 = i * F
        in_maps.append({
            "xT": xT,
            "wq": np.ascontiguousarray(qkv_w[:, c0:c0 + F]),
            "wk": np.ascontiguousarray(qkv_w[:, E + c0:E + c0 + F]),
            "wv": np.ascontiguousarray(qkv_w[:, 2 * E + c0:2 * E + c0 + F]),
            "bq": np.ascontiguousarray(qkv_b[c0:c0 + F].reshape(F, 1)),
            "bk": np.ascontiguousarray(qkv_b[E + c0:E + c0 + F].reshape(F, 1)),
            "bv": np.ascontiguousarray(
                qkv_b[2 * E + c0:2 * E + c0 + F].reshape(F, 1)),
            "ow": o_w,
            "ob": o_b,
        })
    return in_maps


def gather_out(results, B=B_FULL, S=S_FULL):
    """Per-core [B*TPB, E] (TPB tokens per batch) -> full [B, S, E]."""
    TPB = S // N_CORES
    full = np.empty((B, S, E), dtype=np.float32)
    for c in range(N_CORES):
        r = results[c]["out"]
        for b in range(B):
            full[b, c * TPB:(c + 1) * TPB] = r[b * TPB:(b + 1) * TPB]
    return full


_NC_CACHE = {}


def _get_nc(B=B_FULL, S=S_FULL):
    key = (B, S, COMPUTE)
    if key not in _NC_CACHE:
        _NC_CACHE[key] = build_nc(B, S, COMPUTE)
    return _NC_CACHE[key]


def kernel(x, qkv_w, qkv_b, o_w, o_b):
    B, S, _ = np.asarray(x).shape
    nc = _get_nc(B, S)
    in_maps = make_in_maps(x, qkv_w, qkv_b, o_w, o_b, B, S)
    res = bass_utils.run_bass_kernel_spmd(
        nc, in_maps, core_ids=list(range(N_CORES)))
    return gather_out(res.results, B, S)


# revision 41
# speedup vs baseline: 1.0473x; 1.0473x over previous
"""Multi-head attention (B=2, S=2048, E=1024, H=16) on 8 Trainium2 NeuronCores.

Sharding: tensor-parallel over heads — core i owns heads (2i, 2i+1).
Token ownership for the output projection: core i owns tokens
[256*i, 256*i+256) of EACH batch, so the head->token re-shard can be done
with one AllToAll PER BATCH and overlapped with the other batch's compute.

  Phase A  (per core, per batch): q/k/v projections for its 2 heads,
            feature-major into single qfm/kfm tiles (head A on partitions
            0:64, head B on 64:128); v is PE-transposed to token-major with
            a ones column per head (softmax-denominator trick).
  Phase B/C (per core, per batch): scores^T via PE row-tiled K=64 matmul
            pairs at N=512 (head A on array rows 0:64 via tile_position=
            (0,0), head B on rows 64:128 via (64,0) — no zero-padding
            waste); exp straight out of PSUM (no max-subtraction — scores
            are O(1)), split between ScalarE (exact) and DVE (Schraudolph
            bitcast exp into bf16 bit-space, head B on even q-tiles) to
            balance the two engines; AV matmul per 256-col half with the
            ones-row so the denominator falls out of the same fp32
            accumulation (head A's kc=0 start resets the shared PSUM bank,
            zeroing head B's region — bank-wide start semantics); normalize
            via a rank-1 f32r (ones ⊗ den) broadcast matmul +
            reciprocal_approx_fast on the broadcast tile (custom-DVE ops
            only work at base partition 0) — no GpSimd or DMA on the
            per-q-tile path, since the in-flight AllToAll blocks the GpSimd
            queue and would stall it.
  AllToAll: one bf16 collective per batch re-sharding head-parallel
            [128 feat, S tokens] to token-parallel [1024 feat, S/8 tokens].
            Batch 0's collective overlaps phase A/BC of batch 1.
  Phase D  (per core, per batch): output projection for its 256-token slice;
            its PSUM comes from the BC-phase rings so the WAR dependency
            pins the matmuls behind the preceding batch's BC work — the
            scheduler otherwise hoists them into the in-order PE queue where
            they head-of-line block on the collective's output. D(b0)
            overlaps a2a(b1).

Phase A of batch 1 is emitted interleaved into BC(b0)'s q-tiles so the PE
fills ScalarE-bound gaps. Matmuls run in bf16 (full-rate + FWL weight
loads); inputs are cast to bf16 on the host; PSUM accumulation is fp32.

Phase A(0) leads with small (128/256-token) chunks so the first
projection matmuls start after a 256KB xt transfer instead of 1MB.

Measured on 8 axon trn2 cores: ~307-311 us HW exec (mean core ~300-306),
rel err 7.4e-3 (vs 402 us / 3.1e-3 for the padded-K, single-collective,
serial-tail baseline).
"""

import numpy as np
import ml_dtypes

import concourse.bass as bass
import concourse.mybir as mybir
import concourse.tile as tile
from concourse import bacc
from concourse import bass_utils
from concourse.masks import make_identity

F32 = mybir.dt.float32
BF16 = mybir.dt.bfloat16
F32R = mybir.dt.float32r
I16 = mybir.dt.int16
N_CORES = 8
P = 128

COMPUTE = "bf16"
SCORES_TILED = True           # row-tiled K=64 score matmul pairs
EXP_B_DVE = True             # route head-B exp to DVE (Schraudolph bf16)

# Full problem dims (hardcoded per the harness contract)
B_FULL, S_FULL, E, H, D = 2, 2048, 1024, 16, 64
HPC = H // N_CORES            # heads per core = 2
F = HPC * D                   # feature cols per core = 128
SCALE = D ** -0.5

# Schraudolph exp in bf16-bit space: w = bitcast_bf16(int16(round(
#   s * SCALE*log2e*128 + (127<<7) - C)))
LOG2E = 1.4426950408889634
SCH_A = SCALE * LOG2E * 128.0
SCH_B = 16256.0 - 29.0        # assumes round-to-nearest convert


def build_nc(B=B_FULL, S=S_FULL, compute=COMPUTE):
    CDT = BF16 if compute == "bf16" else F32R
    IN_DT = BF16 if compute == "bf16" else F32
    T = B * S                 # tokens
    KO = E // P               # 8 contraction chunks over embed
    TC = min(512, S)          # phase-A token chunk
    NTC = S // TC             # chunks per batch
    Q2 = min(512, S)          # q tile (score matmul N)
    NQ = S // Q2
    QH = min(256, Q2)         # AV/normalize half-tile
    NH = Q2 // QH
    KC = S // P               # k chunks per batch
    G4 = 2                    # kc group per exp call
    TPB = S // N_CORES        # tokens per core PER BATCH
    TT = min(P, TPB)

    nc = bacc.Bacc("TRN2", target_bir_lowering=False, debug=False,
                   num_devices=N_CORES)

    xT = nc.dram_tensor("xT", [E, T], IN_DT, kind="ExternalInput").ap()
    wq = nc.dram_tensor("wq", [E, F], IN_DT, kind="ExternalInput").ap()
    wk = nc.dram_tensor("wk", [E, F], IN_DT, kind="ExternalInput").ap()
    wv = nc.dram_tensor("wv", [E, F], IN_DT, kind="ExternalInput").ap()
    bq = nc.dram_tensor("bq", [F, 1], F32, kind="ExternalInput").ap()
    bk = nc.dram_tensor("bk", [F, 1], F32, kind="ExternalInput").ap()
    bv = nc.dram_tensor("bv", [F, 1], F32, kind="ExternalInput").ap()
    ow = nc.dram_tensor("ow", [E, E], IN_DT, kind="ExternalInput").ap()
    ob = nc.dram_tensor("ob", [1, E], F32, kind="ExternalInput").ap()
    # rows b*TPB+i = batch b, token TPB*core + i
    out = nc.dram_tensor("out", [B * TPB, E], F32, kind="ExternalOutput").ap()

    Exp = mybir.ActivationFunctionType.Exp
    Mult = mybir.AluOpType.mult
    Add = mybir.AluOpType.add

    with tile.TileContext(nc) as tc:
        with tc.tile_pool(name="persist", bufs=1) as persist, \
             tc.tile_pool(name="pAw", bufs=1) as pAw, \
             tc.tile_pool(name="pA", bufs=4) as pA, \
             tc.tile_pool(name="pBC", bufs=2) as pBC, \
             tc.tile_pool(name="pNr", bufs=3) as pNr, \
             tc.tile_pool(name="pD", bufs=1) as pD, \
             tc.tile_pool(name="pDo", bufs=4) as pDo, \
             tc.tile_pool(name="psA", bufs=1, space="PSUM") as psA, \
             tc.tile_pool(name="psS", bufs=2, space="PSUM") as psS, \
             tc.tile_pool(name="psAV", bufs=2, space="PSUM") as psAV, \
             tc.tile_pool(name="psDb", bufs=1, space="PSUM") as psDb, \
             tc.tile_pool(name="dramp", bufs=1, space="DRAM") as dramp:
            ident = persist.tile([P, P], CDT)
            make_identity(nc, ident)
            bq_sb = persist.tile([P, 1], F32)
            bk_sb = persist.tile([P, 1], F32)
            bv_sb = persist.tile([P, 1], F32)
            nc.sync.dma_start(bq_sb, bq)
            nc.sync.dma_start(bk_sb, bk)
            nc.sync.dma_start(bv_sb, bv)
            ob_row = persist.tile([1, E], F32)
            nc.sync.dma_start(ob_row, ob)
            obb = persist.tile([P, E], F32)
            nc.gpsimd.partition_broadcast(obb, ob_row)

            qfm = persist.tile([P, T], CDT)     # q^T (both heads stacked)
            if SCORES_TILED:
                kfm = persist.tile([P, T], CDT)  # k^T (both heads stacked)
            else:
                # zero-padded per head: full-K=128 standard matmuls
                kfmA = persist.tile([P, T], CDT)
                kfmB = persist.tile([P, T], CDT)
                nc.vector.memset(kfmA[64:128], 0.0)
                nc.vector.memset(kfmB[0:64], 0.0)
            # v token-major per 128-token chunk, with a ones column per head:
            # cols 0:64 head A v, col 64 ones(A), 65:129 head B v, col 129
            # ones(B)
            vtm = persist.tile([P, T // P, 130], CDT)
            ones1 = persist.tile([P, 1], F32)
            nc.vector.memset(ones1, 1.0)
            nc.vector.tensor_copy(vtm[:, :, 64], ones1.to_broadcast([P, T // P]))
            nc.vector.tensor_copy(vtm[:, :, 129], ones1.to_broadcast([P, T // P]))

            # f32r constants for the rank-1 normalize matmuls
            ones_f32 = persist.tile([P, 65], F32)
            nc.vector.memset(ones_f32, 1.0)
            ones_bc = persist.tile([P, 65], F32R)
            nc.vector.tensor_copy(ones_bc, ones_f32)

            wq_sb = pAw.tile([P, KO, F], CDT)
            wk_sb = pAw.tile([P, KO, F], CDT)
            wv_sb = pAw.tile([P, KO, F], CDT)
            nc.sync.dma_start(wq_sb, wq.rearrange("(ko p) f -> p ko f", p=P))
            xTr = xT.rearrange("(ko p) t -> p ko t", p=P)
            wkv_loaded = []

            a2a_in = [nc.dram_tensor(f"a2a_in{b}", [N_CORES, P, TPB], CDT,
                                     kind="Internal").ap()
                      for b in range(B)]
            a2a_out = [nc.dram_tensor(f"a2a_out{b}", [N_CORES, P, TPB], CDT,
                                      kind="Internal").ap()
                       for b in range(B)]

            def phase_a_qk(b, t0, tc):
                t0 = b * S + t0
                xt = pA.tile([P, KO, tc], CDT, tag="xt")
                nc.sync.dma_start(xt, xTr[:, :, t0:t0 + tc])
                if not wkv_loaded:
                    nc.sync.dma_start(
                        wk_sb, wk.rearrange("(ko p) f -> p ko f", p=P))
                    nc.sync.dma_start(
                        wv_sb, wv.rearrange("(ko p) f -> p ko f", p=P))
                    wkv_loaded.append(True)
                ps = psA.tile([P, tc], F32, tag="ps")
                for ko in range(KO):
                    nc.tensor.matmul(ps, lhsT=wq_sb[:, ko], rhs=xt[:, ko],
                                     start=(ko == 0), stop=(ko == KO - 1))
                nc.vector.tensor_scalar_add(qfm[:, t0:t0 + tc], ps, bq_sb)
                ps = psA.tile([P, tc], F32, tag="ps")
                for ko in range(KO):
                    nc.tensor.matmul(ps, lhsT=wk_sb[:, ko], rhs=xt[:, ko],
                                     start=(ko == 0), stop=(ko == KO - 1))
                if SCORES_TILED:
                    nc.vector.tensor_scalar_add(kfm[:, t0:t0 + tc], ps, bk_sb)
                else:
                    nc.vector.tensor_scalar_add(kfmA[0:64, t0:t0 + tc],
                                                ps[0:64], bk_sb[0:64])
                    nc.vector.tensor_scalar_add(kfmB[64:128, t0:t0 + tc],
                                                ps[64:128], bk_sb[64:128])
                return xt

            def phase_a_v(b, t0, tc, xt):
                t0 = b * S + t0
                ps = psA.tile([P, tc], F32, tag="ps")
                for ko in range(KO):
                    nc.tensor.matmul(ps, lhsT=wv_sb[:, ko], rhs=xt[:, ko],
                                     start=(ko == 0), stop=(ko == KO - 1))
                vfm = pA.tile([P, tc], CDT, tag="vfm")
                nc.vector.tensor_scalar_add(vfm, ps, bv_sb)
                for sub in range(tc // P):
                    pst = psDb.tile([P, P], CDT, tag="db")
                    nc.tensor.transpose(pst, vfm[:, sub * P:(sub + 1) * P],
                                        ident)
                    c = (t0 + sub * P) // P
                    nc.vector.tensor_copy(vtm[:, c, 0:64], pst[:, 0:64])
                    nc.vector.tensor_copy(vtm[:, c, 65:129], pst[:, 64:128])

            def phase_a(b, chunks=None):
                t0 = 0
                for tc in (chunks or [TC] * NTC):
                    xt = phase_a_qk(b, t0, tc)
                    phase_a_v(b, t0, tc, xt)
                    t0 += tc

            def phase_a_pieces(b):
                t0 = 0
                for tc in [TC] * NTC:
                    xt = phase_a_qk(b, t0, tc)
                    yield
                    phase_a_v(b, t0, tc, xt)
                    yield
                    t0 += tc

            def bc_qtile(b, qi):
                q0 = b * S + qi * Q2
                eA = pBC.tile([P, KC, Q2], CDT, tag="expA")
                eB = pBC.tile([P, KC, Q2], CDT, tag="expB")
                for kg in range(KC // G4):
                    sA = psS.tile([P, G4, Q2], F32, tag="sS")
                    sB = psS.tile([P, G4, Q2], F32, tag="sS")
                    for j in range(G4):
                        kc = kg * G4 + j
                        k0 = b * S + kc * P
                        if SCORES_TILED:
                            nc.tensor.matmul(
                                sA[:, j], lhsT=kfm[0:64, k0:k0 + P],
                                rhs=qfm[0:64, q0:q0 + Q2],
                                start=True, stop=True, tile_position=(0, 0))
                            nc.tensor.matmul(
                                sB[:, j], lhsT=kfm[64:128, k0:k0 + P],
                                rhs=qfm[64:128, q0:q0 + Q2],
                                start=True, stop=True, tile_position=(64, 0))
                        else:
                            nc.tensor.matmul(
                                sA[:, j], lhsT=kfmA[:, k0:k0 + P],
                                rhs=qfm[:, q0:q0 + Q2],
                                start=True, stop=True)
                            nc.tensor.matmul(
                                sB[:, j], lhsT=kfmB[:, k0:k0 + P],
                                rhs=qfm[:, q0:q0 + Q2],
                                start=True, stop=True)
                    g0 = kg * G4
                    nc.scalar.activation(eA[:, g0:g0 + G4], sA, Exp,
                                         scale=SCALE)
                    if EXP_B_DVE and qi % 2 == 0:
                        nc.vector.tensor_scalar(
                            eB[:, g0:g0 + G4].bitcast(I16), sB,
                            SCH_A, SCH_B, op0=Mult, op1=Add)
                    else:
                        nc.scalar.activation(eB[:, g0:g0 + G4], sB, Exp,
                                             scale=SCALE)
                for h in range(NH):
                    bc_avh(b, qi, h, eA, eB)

            def bc_avh(b, qi, h, eA, eB):
                qh0 = h * QH
                pv = psAV.tile([65, 2, QH], F32, tag="av")
                pvA = pv[:, 0]
                pvB = pv[:, 1]
                # chain A's kc=0 start resets the ENTIRE shared PSUM bank
                # (bank-wide start semantics), zeroing chain B's region too;
                # everything after accumulates with start=False
                for kc in range(KC):
                    c = (b * S) // P + kc
                    nc.tensor.matmul(pvA, lhsT=vtm[:, c, 0:65],
                                     rhs=eA[:, kc, qh0:qh0 + QH],
                                     start=(kc == 0), stop=False,
                                     skip_group_check=True)
                    nc.tensor.matmul(pvB, lhsT=vtm[:, c, 65:130],
                                     rhs=eB[:, kc, qh0:qh0 + QH],
                                     start=False, stop=(kc == KC - 1),
                                     skip_group_check=True)
                # normalize: rows 0:64 numerator, row 64 denominator.
                # reciprocal on the psum den rows, then broadcast down 64
                # partitions with a rank-1 (ones ⊗ recip) matmul.
                # gpsimd-free normalize: broadcast the raw denominator rows
                # down 64 partitions with a rank-1 (ones ⊗ den) matmul, then
                # reciprocal on the broadcast tile (custom-DVE ops only work
                # at base partition 0). Keeps the per-q-tile path off the
                # GpSimd queue (the AllToAll blocks that queue in flight).
                rr2 = pNr.tile([65, 2, QH], F32R, tag="rr2")
                nc.vector.tensor_copy(rr2[64:65], pv[64:65])
                db = psDb.tile([64, 2, QH], F32, tag="db")
                nc.tensor.matmul(db,
                                 lhsT=ones_bc[64:65, 0:64],
                                 rhs=rr2[64:65],
                                 start=True, stop=True,
                                 tile_position=(64, 0))
                dbs = pNr.tile([64, 2, QH], F32, tag="dbs")
                nc.vector.tensor_copy(dbs, db)
                nc.vector.reciprocal_approx_fast(dbs, dbs)
                stgA = pNr.tile([64, QH], CDT, tag="stgA")
                stgB = pNr.tile([64, QH], CDT, tag="stgB")
                nc.vector.tensor_mul(stgA, pvA[0:64], dbs[:, 0])
                nc.vector.tensor_mul(stgB, pvB[0:64], dbs[:, 1])
                dest = qi * NH + h
                nc.sync.dma_start(a2a_in[b][dest, 0:64], stgA)
                nc.sync.dma_start(a2a_in[b][dest, 64:128], stgB)

            def send_a2a(b):
                nc.gpsimd.collective_compute(
                    "AllToAll", mybir.AluOpType.bypass,
                    replica_groups=[list(range(N_CORES))],
                    ins=[a2a_in[b].opt()], outs=[a2a_out[b].opt()])

            ow_sb = pD.tile([P, KO, E], CDT)

            def d_block(ga, tslice, row0, nrows):
                # output-projection for ga[:, :, tslice] -> out rows
                # [row0, row0+nrows); PSUM from the BC rings so the WAR dep
                # pins these matmuls behind the preceding BC work in the
                # in-order PE queue (the scheduler otherwise hoists them to
                # block head-of-line on the collective's output)
                for n2 in range(E // 512):
                    if n2 % 2 == 0:
                        pso = psDb.tile([P, 512], F32, tag="db")
                    else:
                        pso = psS.tile([P, 512], F32, tag="sS")
                    for r in range(N_CORES):
                        nc.tensor.matmul(
                            pso[0:nrows],
                            lhsT=ga[:, r, tslice],
                            rhs=ow_sb[:, r, n2 * 512:(n2 + 1) * 512],
                            start=(r == 0), stop=(r == N_CORES - 1))
                    osb = pDo.tile([TT, 512], F32, tag="osb")
                    nc.vector.tensor_add(osb[0:nrows], pso[0:nrows],
                                         obb[0:nrows,
                                             n2 * 512:(n2 + 1) * 512])
                    nc.sync.dma_start(
                        out[row0:row0 + nrows, n2 * 512:(n2 + 1) * 512],
                        osb[0:nrows])

            def phase_d(b):
                ga = pD.tile([P, N_CORES, TPB], CDT, name=f"ga{b}")
                nc.sync.dma_start(ga, a2a_out[b].rearrange("c p t -> p c t"))
                for t4 in range(TPB // TT):
                    d_block(ga, slice(t4 * TT, (t4 + 1) * TT),
                            b * TPB + t4 * TT, TT)

            # small leading chunks: the first q-projection matmuls start
            # after a 256KB xt transfer instead of 1MB
            phase_a(0, chunks=[128, 128, 256, 512, 512, 512])
            if B > 1:
                # interleave A(b1) pieces into BC(b0)'s scalar-bound q-tiles
                a1 = phase_a_pieces(1)
                for qi in range(NQ):
                    bc_qtile(0, qi)
                    next(a1, None)
                    next(a1, None)
                send_a2a(0)
                nc.sync.dma_start(ow_sb,
                                  ow.rearrange("(r p) e -> p r e", p=P))
                for qi in range(NQ):
                    bc_qtile(1, qi)
                phase_d(0)
                send_a2a(1)
                phase_d(1)
            else:
                for qi in range(NQ):
                    bc_qtile(0, qi)
                send_a2a(0)
                nc.sync.dma_start(ow_sb,
                                  ow.rearrange("(r p) e -> p r e", p=P))
                phase_d(0)

    nc.compile()
    return nc


def make_in_maps(x, qkv_w, qkv_b, o_w, o_b, B=B_FULL, S=S_FULL,
                 compute=COMPUTE):
    """Host-side sharding: full inputs -> per-core input dicts."""
    T = B * S
    idt = ml_dtypes.bfloat16 if compute == "bf16" else np.float32
    x = np.asarray(x, dtype=np.float32)
    qkv_w = np.asarray(qkv_w, dtype=np.float32).astype(idt)
    qkv_b = np.asarray(qkv_b, dtype=np.float32)
    o_w = np.ascontiguousarray(np.asarray(o_w, dtype=np.float32).astype(idt))
    o_b = np.asarray(o_b, dtype=np.float32).reshape(1, E)
    xT = np.ascontiguousarray(x.reshape(T, E).T.astype(idt))
    in_maps = []
    for i in range(N_CORES):
        c0 = i * F
        in_maps.append({
            "xT": xT,
            "wq": np.ascontiguousarray(qkv_w[:, c0:c0 + F]),
            "wk": np.ascontiguousarray(qkv_w[:, E + c0:E + c0 + F]),
            "wv": np.ascontiguousarray(qkv_w[:, 2 * E + c0:2 * E + c0 + F]),
            "bq": np.ascontiguousarray(qkv_b[c0:c0 + F].reshape(F, 1)),
            "bk": np.ascontiguousarray(qkv_b[E + c0:E + c0 + F].reshape(F, 1)),
            "bv": np.ascontiguousarray(
                qkv_b[2 * E + c0:2 * E + c0 + F].reshape(F, 1)),
            "ow": o_w,
            "ob": o_b,
        })
    return in_maps


def gather_out(results, B=B_FULL, S=S_FULL):
    """Per-core [B*TPB, E] (TPB tokens per batch) -> full [B, S, E]."""
    TPB = S // N_CORES
    full = np.empty((B, S, E), dtype=np.float32)
    for c in range(N_CORES):
        r = results[c]["out"]
        for b in range(B):
            full[b, c * TPB:(c + 1) * TPB] = r[b * TPB:(b + 1) * TPB]
    return full


_NC_CACHE = {}


def _get_nc(B=B_FULL, S=S_FULL):
    key = (B, S, COMPUTE)
    if key not in _NC_CACHE:
        _NC_CACHE[key] = build_nc(B, S, COMPUTE)
    return _NC_CACHE[key]


def kernel(x, qkv_w, qkv_b, o_w, o_b):
    B, S, _ = np.asarray(x).shape
    nc = _get_nc(B, S)
    in_maps = make_in_maps(x, qkv_w, qkv_b, o_w, o_b, B, S)
    res = bass_utils.run_bass_kernel_spmd(
        nc, in_maps, core_ids=list(range(N_CORES)))
    return gather_out(res.results, B, S)
